# revision 20
# baseline (speedup 1.0000x reference)
"""Trainium2 Bass kernel for nn_CtoX (gnn_message_passing).

Computes, per batch b:
  stage1 (CtoE): block-pair stats (mean/min/max/std with pairwise masks) of
     delta1[b] over 16x16 atom blocks -> z[b, 16, 16, 256] -> E = z @ W1.T + b1
  stage2 (EtoX): masked stats of E over its second block axis -> zE[b,16,256]
     -> out = zE @ W2.T + b2   (out: [4, 16, 256])

Sharding: 8 cores = (4 batches) x (2 halves of the first nm axis).
Each core handles delta1[b, ih*128:(ih+1)*128, :, :] (8 MiB) and produces
out[b, ih*8:(ih+1)*8, :] with zero cross-core communication.

v3 layout notes:
  - The host pre-transposes each core's delta1 slice to [128 i, J, c, a]
    (c outer, a inner) so that on-device the bias-add and the grouped
    min/max reduce over `a` are all inner-contiguous (2 elem/cycle DVE
    path) and the per-J DMA lands contiguously (1 descriptor/partition).
  - Per-J working tile dsq[P, 128, MA]: columns 0:64 = d (DMA),
    64:128 = d^2 (Scalar Square).  The sums matmul uses the dsq column
    for one `a` as the STATIONARY tensor and the 16 indicator columns
    [bind*cm1*cm2[j] | bind] as the MOVING tensor, so the accumulated
    result S_psT[(c|c^2), J, (Sm I | S1 I)] comes out with features on
    partitions -- no stage-2 sum transposes needed at all.
  - Bias-adds run on GpSimd for most J (Vector keeps all reduces since
    GpSimd can't reduce over free axes); all mask-derived constants are
    host-precomputed into one cpack tensor.
"""

import numpy as np
from contextlib import ExitStack

BIG = 100000.0
EPS = 1e-8

D_C = 64      # channel dim of delta1
D_X = 256     # output feature dim
MA = 16       # atoms per block
P = 128       # partitions per core (half of nm)
NI = 8        # I-blocks per core
NJ = 16       # J-blocks
NM = 256

GMIN = 6      # J < GMIN: min-path bias-add on GpSimd (else Vector).
              # Max-path bias-adds all run on GpSimd.

# cpack column offsets (one packed [128, CPACK_COLS] constants tensor)
# -- section A (loop-critical; split into several DMAs across queues) --
OFF_BIASF = 0                      # [256] BIG*(1-cm1*cm2)
OFF_BIASFN = OFF_BIASF + NM        # [256] -BIG*(1-cm1*cm2)
OFF_LHST = OFF_BIASFN + NM         # [256*16] lhsT fields per j
A_COLS = OFF_LHST + NM * 16
# -- section B (stage-2, last DMA) --
OFF_IDENT = A_COLS                 # [128]
OFF_RECIPD = OFF_IDENT + P         # [8*16] 1/(cnt1*cnt2+eps), (I,J) order
OFF_NFAC = OFF_RECIPD + NJ * NI    # [8*16] 1-EPS/div, (I,J) order
OFF_W1T = OFF_NFAC + NJ * NI       # [256]
OFF_W2T = OFF_W1T + 256            # [512]
OFF_B1 = OFF_W2T + 512             # [1]
OFF_B2 = OFF_B1 + 1                # [2]
OFF_EM = OFF_B2 + 2                # [16]
OFF_BIASE = OFF_EM + NJ            # [16]
OFF_BIASEN = OFF_BIASE + NJ        # [16]
OFF_RECIPE = OFF_BIASEN + NJ       # [1]
CPACK_COLS = OFF_RECIPE + 1

_CACHE = {}


def _build_program():
    import concourse.bass as bass
    import concourse.bacc as bacc
    import concourse.tile as tile
    import concourse.mybir as mybir

    f32 = mybir.dt.float32
    Alu = mybir.AluOpType
    Act = mybir.ActivationFunctionType
    AX = mybir.AxisListType

    nc = bacc.Bacc()

    # d, host-pretransposed: [i, J, c, a] flattened to [128, NJ, D_C*MA]
    d_in = nc.dram_tensor("d", [P, NJ, D_C * MA], f32, kind="ExternalInput")
    cpack_in = nc.dram_tensor("cpack", [P, CPACK_COLS], f32, kind="ExternalInput")
    out_t = nc.dram_tensor("out_t", [D_X, NI], f32, kind="ExternalOutput")

    with tile.TileContext(nc) as tc, ExitStack() as ctx:
        consts = ctx.enter_context(tc.tile_pool(name="consts", bufs=1))
        small = ctx.enter_context(tc.tile_pool(name="small", bufs=1))

        # ---------- constant loads: split across DMA queues so the
        # loop-critical constants (biases first, then lhsT chunks in loop
        # order) land fast instead of serializing on one queue ----------
        cpak = consts.tile([P, CPACK_COLS], f32)
        nc.sync.dma_start(
            out=cpak[:, 0:OFF_LHST], in_=cpack_in[:, 0:OFF_LHST]
        )
        LCH = NM * 16 // 4
        for c in range(4):
            nc.sync.dma_start(
                out=cpak[:, OFF_LHST + c * LCH : OFF_LHST + (c + 1) * LCH],
                in_=cpack_in[:, OFF_LHST + c * LCH : OFF_LHST + (c + 1) * LCH],
            )
        nc.sync.dma_start(
            out=cpak[:, A_COLS:CPACK_COLS], in_=cpack_in[:, A_COLS:CPACK_COLS]
        )
        lhsTs = cpak[:, OFF_LHST : OFF_LHST + NM * 16].rearrange(
            "p (j k) -> p j k", k=16
        )
        biasF = cpak[:, OFF_BIASF : OFF_BIASF + NM]
        biasFn = cpak[:, OFF_BIASFN : OFF_BIASFN + NM]
        ident = cpak[:, OFF_IDENT : OFF_IDENT + P]
        recipD = cpak[:, OFF_RECIPD : OFF_RECIPD + NI * NJ].rearrange(
            "p (I J) -> p I J", J=NJ
        )
        nfac = cpak[:, OFF_NFAC : OFF_NFAC + NI * NJ].rearrange(
            "p (I J) -> p I J", J=NJ
        )
        w1t_a = cpak[:, OFF_W1T : OFF_W1T + 128]
        w1t_b = cpak[:, OFF_W1T + 128 : OFF_W1T + 256]
        w2t_a = cpak[:, OFF_W2T : OFF_W2T + 256]
        w2t_b = cpak[:, OFF_W2T + 256 : OFF_W2T + 512]
        b1c = cpak[:, OFF_B1 : OFF_B1 + 1]
        b2c_a = cpak[:, OFF_B2 : OFF_B2 + 1]
        b2c_b = cpak[:, OFF_B2 + 1 : OFF_B2 + 2]
        emrep = cpak[:, OFF_EM : OFF_EM + NJ]
        biasE = cpak[:, OFF_BIASE : OFF_BIASE + NJ]
        biasEn = cpak[:, OFF_BIASEN : OFF_BIASEN + NJ]
        recipE = cpak[:, OFF_RECIPE : OFF_RECIPE + 1]

        # ---------- big J-loop ----------
        # umm[:, J, 0:64] = per-(i, J, c) biased max; [:, J, 64:128] = biased
        # min -- packed so ONE transpose per J lands ma-feats at partitions
        # 0:64 and mi-feats at 64:128.
        umm = consts.tile([P, NJ, P], f32)
        # z matrices in [feature, row=(I,J)] layout:
        #   rhs_z0: [0:64] m-feats, [64:128] mi-feats
        #   rhs_z1: [0:64] ma-feats, [64:128] std-feats
        rhs_z0 = small.tile([P, P], f32)
        rhs_z1 = small.tile([P, P], f32)
        Ssb = small.tile([16, NJ, P], f32)
        S2sb = small.tile([8, NJ, P], f32)

        with tc.tile_pool(name="psum_sums", bufs=1, space="PSUM") as psum_sums, \
             tc.tile_pool(name="psum_tr", bufs=2, space="PSUM") as psum_tr, \
             tc.tile_pool(name="loop", bufs=3) as loop_pool, \
             tc.tile_pool(name="btmp", bufs=3) as btmp_pool, \
             tc.tile_pool(name="gtmp", bufs=3) as gtmp_pool:
            S_ps = psum_sums.tile([16, NJ, P], f32)

            for J in range(NJ):
                # packed [d | d^2] tile, layout [i, c2, a] (a contiguous)
                dsq = loop_pool.tile([P, P, MA], f32, tag="dsq")
                nc.sync.dma_start(
                    out=dsq[:, 0:64, :].rearrange("p c a -> p (c a)"),
                    in_=d_in[:, J, :],
                )
                nc.scalar.activation(
                    out=dsq[:, 64:128, :], in_=dsq[:, 0:64, :], func=Act.Square
                )

                # min path: bias-add on GpSimd for J < GMIN, else Vector
                menge = nc.gpsimd if J < GMIN else nc.vector
                mpool = gtmp_pool if J < GMIN else btmp_pool
                bt = mpool.tile([P, D_C, MA], f32, tag="bt")
                menge.tensor_tensor(
                    out=bt,
                    in0=dsq[:, 0:64, :],
                    in1=biasF[:, J * MA : (J + 1) * MA]
                    .unsqueeze(1)
                    .broadcast_to([P, D_C, MA]),
                    op=Alu.add,
                )
                nc.vector.tensor_reduce(
                    out=umm[:, J, 64:128],
                    in_=bt[:],
                    axis=AX.X,
                    op=Alu.min,
                )
                # max path: bias-add on GpSimd for all J
                bt2 = gtmp_pool.tile([P, D_C, MA], f32, tag="bt2")
                nc.gpsimd.tensor_tensor(
                    out=bt2,
                    in0=dsq[:, 0:64, :],
                    in1=biasFn[:, J * MA : (J + 1) * MA]
                    .unsqueeze(1)
                    .broadcast_to([P, D_C, MA]),
                    op=Alu.add,
                )
                nc.vector.tensor_reduce(
                    out=umm[:, J, 0:64],
                    in_=bt2[:],
                    axis=AX.X,
                    op=Alu.max,
                )

                # sums: ONE matmul per j, lhsT = [bind*cm1*cm2[j] | bind]
                # (16 cols), rhs = dsq[:, :, jj] = the [d | d^2] column for
                # this a (128 strided cols).  Rows 0:8 = Sm (cols 0:64) and
                # S2 (cols 64:128); rows 8:16 = S1 (cols 0:64).
                for jj in range(MA):
                    j = J * MA + jj
                    nc.tensor.matmul(
                        S_ps[:, J, :],
                        lhsT=lhsTs[:, j, :],
                        rhs=dsq[:, :, jj],
                        start=(jj == 0),
                        stop=(jj == MA - 1),
                    )

            # evacuate sums, duplicated into both 64-column halves so one
            # transpose serves lo and hi features.
            nc.scalar.copy(Ssb[:, :, 0:64], S_ps[0:16, :, 0:64])
            nc.scalar.copy(Ssb[:, :, 64:128], S_ps[0:16, :, 0:64])
            nc.scalar.copy(S2sb[:, :, 0:64], S_ps[0:8, :, 64:128])
            nc.scalar.copy(S2sb[:, :, 64:128], S_ps[0:8, :, 64:128])

            # min/max stage 2: one transpose per J into a PSUM ring of 4,
            # then ONE batched grouped reduce per 4-J round per feature-half.
            for Jr in range(0, NJ, 4):
                TP = psum_tr.tile([P, 4, P], f32, tag="tp")
                for k in range(4):
                    nc.tensor.transpose(
                        out=TP[:, k, :], in_=umm[:, Jr + k, :], identity=ident
                    )
                nc.vector.tensor_reduce(
                    out=rhs_z0[64:128, :]
                    .rearrange("p (I J) -> p J I", J=NJ)[:, Jr : Jr + 4, :],
                    in_=TP[64:128, :, :].rearrange("p r (I a) -> p r I a", a=MA),
                    axis=AX.X,
                    op=Alu.min,
                )
                nc.vector.tensor_reduce(
                    out=rhs_z1[0:64, :]
                    .rearrange("p (I J) -> p J I", J=NJ)[:, Jr : Jr + 4, :],
                    in_=TP[0:64, :, :].rearrange("p r (I a) -> p r I a", a=MA),
                    axis=AX.X,
                    op=Alu.max,
                )

        # ---------- stage 2: sums transposes to [feature, row] layout ----
        with tc.tile_pool(name="psum_ts", bufs=1, space="PSUM") as psum_ts, \
             tc.tile_pool(name="psum_e", bufs=1, space="PSUM") as psum_e, \
             tc.tile_pool(name="psum_o", bufs=1, space="PSUM") as psum_o:
            # sums: one transpose per J: [16, 128(dup)] -> [128, 16]
            # (columns = [SmT | S1T]); rows 0:64 serve m, 64:128 serve std.
            SST_ps = psum_ts.tile([P, 16, NJ], f32, tag="sst")
            S2T_ps = psum_ts.tile([P, NI, NJ], f32, tag="s2t")
            for J in range(NJ):
                nc.tensor.transpose(
                    out=SST_ps[:, :, J],
                    in_=Ssb[0:16, J, :],
                    identity=ident[0:16, 0:16],
                )
                nc.tensor.transpose(
                    out=S2T_ps[:, :, J],
                    in_=S2sb[0:8, J, :],
                    identity=ident[0:8, 0:8],
                )

            SST = small.tile([P, 16, NJ], f32)
            nc.scalar.copy(SST[:], SST_ps[:])
            S2T = small.tile([P, NI, NJ], f32)
            nc.scalar.copy(S2T[64:128], S2T_ps[64:128])
            SmT = SST[:, 0:8, :]
            S1T = SST[:, 8:16, :]

            # m = S1/div  (lo half -> m-features; hi half feeds std)
            mT = small.tile([P, NI, NJ], f32)
            nc.vector.tensor_tensor(out=mT[:], in0=S1T, in1=recipD, op=Alu.mult)
            nc.vector.tensor_copy(
                out=rhs_z0[0:64, :].rearrange("p (I J) -> p I J", J=NJ),
                in_=mT[0:64],
            )
            # std = S2/div - 2*m*(Sm/div) + m^2*nfac     (hi half only)
            A = small.tile([P, NI, NJ], f32)
            nc.vector.tensor_tensor(
                out=A[64:128], in0=S2T[64:128], in1=recipD[64:128], op=Alu.mult
            )
            Bq = small.tile([P, NI, NJ], f32)
            nc.vector.tensor_tensor(
                out=Bq[64:128], in0=SmT[64:128], in1=recipD[64:128], op=Alu.mult
            )
            nc.vector.tensor_tensor(
                out=Bq[64:128], in0=Bq[64:128], in1=mT[64:128], op=Alu.mult
            )
            nc.vector.tensor_scalar(
                Bq[64:128], Bq[64:128], -2.0, None, Alu.mult
            )  # -2*m*Sm/div
            nc.vector.tensor_tensor(
                out=A[64:128], in0=A[64:128], in1=Bq[64:128], op=Alu.add
            )
            Cq = small.tile([P, NI, NJ], f32)
            nc.vector.tensor_tensor(
                out=Cq[64:128], in0=mT[64:128], in1=mT[64:128], op=Alu.mult
            )
            nc.vector.tensor_tensor(
                out=Cq[64:128], in0=Cq[64:128], in1=nfac[64:128], op=Alu.mult
            )
            nc.vector.tensor_tensor(
                out=rhs_z1[64:128, :].rearrange("p (I J) -> p I J", J=NJ),
                in0=A[64:128],
                in1=Cq[64:128],
                op=Alu.add,
            )

            # ---------- E = z @ W1.T + b1 (dup channels on 128 parts) ----
            E_ps = psum_e.tile([P, P], f32)
            nc.tensor.matmul(
                E_ps[:], lhsT=w1t_a, rhs=rhs_z0[:], start=True, stop=False
            )
            nc.tensor.matmul(
                E_ps[:], lhsT=w1t_b, rhs=rhs_z1[:], start=False, stop=True
            )
            E_T = small.tile([P, P], f32)  # [128(dup chan), 128 rows=(I,J)]
            nc.scalar.activation(
                out=E_T[:], in_=E_ps[:], func=Act.Identity, bias=b1c, scale=1.0
            )

            # ---------- stage 2 of the net: masked stats over J ----------
            E_r = E_T[:].rearrange("p (I J) -> p I J", J=NJ)
            zE0 = small.tile([P, NI], f32)  # [0:64] mE, [64:128] miE
            zE1 = small.tile([P, NI], f32)  # [0:64] maE, [64:128] stdE

            # mE (all partitions; lo half is the m-feature, hi feeds stdE)
            mE = small.tile([P, NI], f32)
            nc.vector.tensor_reduce(out=mE[:], in_=E_r, axis=AX.X, op=Alu.add)
            nc.scalar.mul(mE[:], mE[:], recipE)
            nc.scalar.copy(zE0[0:64, :], mE[0:64, :])

            # miE on hi half
            bE = small.tile([P, NI, NJ], f32)
            nc.gpsimd.tensor_tensor(
                out=bE[64:128],
                in0=E_r[64:128],
                in1=biasE[64:128].unsqueeze(1).broadcast_to([64, NI, NJ]),
                op=Alu.add,
            )
            nc.vector.tensor_reduce(
                out=zE0[64:128, :], in_=bE[64:128], axis=AX.X, op=Alu.min
            )
            # maE on lo half (Vector)
            bE2 = small.tile([P, NI, NJ], f32)
            nc.vector.tensor_tensor(
                out=bE2[0:64],
                in0=E_r[0:64],
                in1=biasEn[0:64].unsqueeze(1).broadcast_to([64, NI, NJ]),
                op=Alu.add,
            )
            nc.vector.tensor_reduce(
                out=zE1[0:64, :], in_=bE2[0:64], axis=AX.X, op=Alu.max
            )
            # stdE on hi half: sum(em*(E-mE)^2)/denom  (GpSimd)
            dev = small.tile([P, NI, NJ], f32)
            nc.gpsimd.tensor_tensor(
                out=dev[64:128],
                in0=E_r[64:128],
                in1=mE[64:128].unsqueeze(2).broadcast_to([64, NI, NJ]),
                op=Alu.subtract,
            )
            nc.gpsimd.tensor_tensor(
                out=dev[64:128], in0=dev[64:128], in1=dev[64:128], op=Alu.mult
            )
            nc.gpsimd.tensor_tensor(
                out=dev[64:128],
                in0=dev[64:128],
                in1=emrep[64:128].unsqueeze(1).broadcast_to([64, NI, NJ]),
                op=Alu.mult,
            )
            nc.vector.tensor_reduce(
                out=zE1[64:128, :], in_=dev[64:128], axis=AX.X, op=Alu.add
            )
            nc.scalar.mul(zE1[64:128, :], zE1[64:128, :], recipE[64:128])

            # ---------- out = zE @ W2.T + b2 ----------
            outa_ps = psum_o.tile([128, NI], f32)
            outb_ps = psum_o.tile([128, NI], f32)
            nc.tensor.matmul(
                outa_ps[:], lhsT=w2t_a[:, 0:128], rhs=zE0[:], start=True, stop=False
            )
            nc.tensor.matmul(
                outa_ps[:], lhsT=w2t_b[:, 0:128], rhs=zE1[:], start=False, stop=True
            )
            nc.tensor.matmul(
                outb_ps[:], lhsT=w2t_a[:, 128:256], rhs=zE0[:], start=True, stop=False
            )
            nc.tensor.matmul(
                outb_ps[:], lhsT=w2t_b[:, 128:256], rhs=zE1[:], start=False,
                stop=True,
            )
            outa = small.tile([128, NI], f32)
            nc.scalar.activation(
                out=outa[:], in_=outa_ps[:], func=Act.Identity, bias=b2c_a,
                scale=1.0,
            )
            outb = small.tile([128, NI], f32)
            nc.scalar.activation(
                out=outb[:], in_=outb_ps[:], func=Act.Identity, bias=b2c_b,
                scale=1.0,
            )
            nc.sync.dma_start(out=out_t[0:128, :], in_=outa[:])
            nc.sync.dma_start(out=out_t[128:256, :], in_=outb[:])

    nc.finalize()  # Bacc: runs compile() (wait splitting, reg alloc, ...)
    return nc


def _get_program():
    if "nc" not in _CACHE:
        _CACHE["nc"] = _build_program()
    return _CACHE["nc"]


def _make_in_maps(delta1, c_mask1, c_mask2, e_mask2, W1, b1, W2, b2):
    delta1 = np.asarray(delta1, dtype=np.float32)
    c_mask1 = np.asarray(c_mask1, dtype=np.float32)
    c_mask2 = np.asarray(c_mask2, dtype=np.float32)
    e_mask2 = np.asarray(e_mask2, dtype=np.float32)
    W1 = np.asarray(W1, dtype=np.float32)
    b1 = np.asarray(b1, dtype=np.float32)
    W2 = np.asarray(W2, dtype=np.float32)
    b2 = np.asarray(b2, dtype=np.float32)

    w1t = np.concatenate([W1.T, W1.T], axis=1)  # [256, 128] (dup out-chan)
    w2t = W2.T  # [256, 256]
    bindm = np.zeros((128, 8), dtype=np.float32)
    for i in range(128):
        bindm[i, i // 16] = 1.0
    identm = np.eye(128, dtype=np.float32)

    in_maps = []
    for k in range(8):
        b, ih = k // 2, k % 2
        cm1 = c_mask1[b, ih * 128 : (ih + 1) * 128, 0, 0]        # [128]
        cm2 = c_mask2[b, 0, :, 0]                                 # [256]
        em = e_mask2[b, 0, :, 0]                                  # [16]

        # pre-transpose d to [i, J, c, a]
        dslab = delta1[b, ih * 128 : (ih + 1) * 128]              # [128,256,64]
        dT = np.ascontiguousarray(
            dslab.reshape(128, NJ, MA, D_C).transpose(0, 1, 3, 2)
        ).reshape(128, NJ, D_C * MA)

        cp = np.zeros((128, CPACK_COLS), dtype=np.float32)
        # lhsT fields: [128, 256 j, 16]: cols 0:8 bind*cm1*cm2[j], 8:16 bind
        lhst = np.zeros((128, NM, 16), dtype=np.float32)
        lhst[:, :, 0:8] = (
            bindm[:, None, :] * cm1[:, None, None] * cm2[None, :, None]
        )
        lhst[:, :, 8:16] = bindm[:, None, :]
        cp[:, OFF_LHST : OFF_LHST + NM * 16] = lhst.reshape(128, NM * 16)
        t2 = cm1[:, None] * cm2[None, :]                          # [128, 256]
        cp[:, OFF_BIASF : OFF_BIASF + NM] = BIG * (1.0 - t2)
        cp[:, OFF_BIASFN : OFF_BIASFN + NM] = -BIG * (1.0 - t2)
        cp[:, OFF_IDENT : OFF_IDENT + P] = identm
        cnt1 = bindm.T @ cm1                                      # [8]
        cnt2 = cm2.reshape(NJ, MA).sum(axis=1)                    # [16]
        div = cnt1[:, None] * cnt2[None, :] + EPS                 # [8 I, 16 J]
        cp[:, OFF_RECIPD : OFF_RECIPD + NJ * NI] = (1.0 / div).reshape(-1)[None, :]
        cp[:, OFF_NFAC : OFF_NFAC + NJ * NI] = (1.0 - EPS / div).reshape(-1)[None, :]
        cp[:, OFF_W1T : OFF_W1T + 128] = w1t[0:128, :]
        cp[:, OFF_W1T + 128 : OFF_W1T + 256] = w1t[128:256, :]
        cp[:, OFF_W2T : OFF_W2T + 256] = w2t[0:128, :]
        cp[:, OFF_W2T + 256 : OFF_W2T + 512] = w2t[128:256, :]
        cp[:, OFF_B1] = np.concatenate([b1, b1])
        cp[:, OFF_B2] = b2[0:128]
        cp[:, OFF_B2 + 1] = b2[128:256]
        cp[:, OFF_EM : OFF_EM + NJ] = em[None, :]
        cp[:, OFF_BIASE : OFF_BIASE + NJ] = (BIG * (1.0 - em))[None, :]
        cp[:, OFF_BIASEN : OFF_BIASEN + NJ] = (-BIG * (1.0 - em))[None, :]
        cp[:, OFF_RECIPE] = 1.0 / em.sum()
        in_maps.append(dict(d=dT, cpack=cp))
    return in_maps


def _assemble(results):
    out = np.empty((4, 16, 256), dtype=np.float32)
    for k in range(8):
        b, ih = k // 2, k % 2
        out[b, ih * 8 : (ih + 1) * 8, :] = results[k]["out_t"].T
    return out


def run(trace=False, **inputs):
    from concourse.bass_utils import run_bass_kernel_spmd

    nc = _get_program()
    in_maps = _make_in_maps(**inputs)
    res = run_bass_kernel_spmd(
        nc, in_maps, core_ids=list(range(8)), trace=trace
    )
    return _assemble(res.results), res


def kernel(**inputs):
    out, _ = run(trace=False, **inputs)
    return out


# revision 29
# speedup vs baseline: 1.3607x; 1.3607x over previous
"""Trainium2 Bass kernel for nn_CtoX (gnn_message_passing).

Computes, per batch b:
  stage1 (CtoE): block-pair stats (mean/min/max/std with pairwise masks) of
     delta1[b] over 16x16 atom blocks -> z[b, 16, 16, 256] -> E = z @ W1.T + b1
  stage2 (EtoX): masked stats of E over its second block axis -> zE[b,16,256]
     -> out = zE @ W2.T + b2   (out: [4, 16, 256])

Sharding: 8 cores = (4 batches) x (2 halves of the first nm axis).
Each core handles delta1[b, ih*128:(ih+1)*128, :, :] (8 MiB) and produces
out[b, ih*8:(ih+1)*8, :] with zero cross-core communication.

v3 layout notes:
  - The host pre-transposes each core's delta1 slice to [128 i, J, c, a]
    (c outer, a inner) so that on-device the bias-add and the grouped
    min/max reduce over `a` are all inner-contiguous (2 elem/cycle DVE
    path) and the per-J DMA lands contiguously (1 descriptor/partition).
  - Per-J working tile dsq[P, 128, MA]: columns 0:64 = d (DMA),
    64:128 = d^2 (Scalar Square).  The sums matmul uses the dsq column
    for one `a` as the STATIONARY tensor and the 16 indicator columns
    [bind*cm1*cm2[j] | bind] as the MOVING tensor, so the accumulated
    result S_psT[(c|c^2), J, (Sm I | S1 I)] comes out with features on
    partitions -- no stage-2 sum transposes needed at all.
  - Bias-adds run on GpSimd for most J (Vector keeps all reduces since
    GpSimd can't reduce over free axes); all mask-derived constants are
    host-precomputed into one cpack tensor.
"""

import numpy as np
import jax.numpy as jnp
from contextlib import ExitStack

BIG = 100000.0
EPS = 1e-8

D_C = 64      # channel dim of delta1
D_X = 256     # output feature dim
MA = 16       # atoms per block
P = 128       # partitions per core (half of nm)
NI = 8        # I-blocks per core
NJ = 16       # J-blocks
NM = 256

GMIN = 6      # J < GMIN: min-path bias-add on GpSimd (else Vector).
              # Max-path bias-adds all run on GpSimd.

# cpack column offsets (one packed [128, CPACK_COLS] constants tensor)
# -- section A (loop-critical first DMA; bf16 lhsT ships separately) --
OFF_BIASF = 0                      # [256] BIG*(1-cm1*cm2)
OFF_BIASFN = OFF_BIASF + NM        # [256] -BIG*(1-cm1*cm2)
A_COLS = OFF_BIASFN + NM
# -- section B (stage-2, last DMA) --
OFF_IDENT = A_COLS                 # [128]
OFF_RECIPD = OFF_IDENT + P         # [8*16] 1/(cnt1*cnt2+eps), (I,J) order
OFF_NFAC = OFF_RECIPD + NJ * NI    # [8*16] 1-EPS/div, (I,J) order
OFF_W1T = OFF_NFAC + NJ * NI       # [256]
OFF_W2T = OFF_W1T + 256            # [512]
OFF_B1 = OFF_W2T + 512             # [1]
OFF_B2 = OFF_B1 + 1                # [2]
OFF_EM = OFF_B2 + 2                # [16]
OFF_BIASE = OFF_EM + NJ            # [16]
OFF_BIASEN = OFF_BIASE + NJ        # [16]
OFF_RECIPE = OFF_BIASEN + NJ       # [1]
CPACK_COLS = OFF_RECIPE + 1

_CACHE = {}


def _build_program():
    import concourse.bass as bass
    import concourse.bacc as bacc
    import concourse.tile as tile
    import concourse.mybir as mybir

    f32 = mybir.dt.float32
    bf16 = mybir.dt.bfloat16
    Alu = mybir.AluOpType
    Act = mybir.ActivationFunctionType
    AX = mybir.AxisListType

    nc = bacc.Bacc()

    # d, host-pretransposed: [i, J, c, a] flattened to [128, NJ, D_C*MA]
    d_in = nc.dram_tensor("d", [P, NJ, D_C * MA], f32, kind="ExternalInput")
    cpack_in = nc.dram_tensor("cpack", [P, CPACK_COLS], f32, kind="ExternalInput")
    # bf16 copy of the lhsT indicator fields (values in {0,1} exact in bf16)
    lhstb_in = nc.dram_tensor("lhstb", [P, NM * 16], bf16, kind="ExternalInput")
    out_t = nc.dram_tensor("out_t", [D_X, NI], f32, kind="ExternalOutput")

    with tile.TileContext(nc) as tc, ExitStack() as ctx:
        consts = ctx.enter_context(tc.tile_pool(name="consts", bufs=1))
        small = ctx.enter_context(tc.tile_pool(name="small", bufs=1))

        # ---------- constant loads: split across DMA queues so the
        # loop-critical constants (biases first, then lhsT chunks in loop
        # order) land fast instead of serializing on one queue ----------
        cpak = consts.tile([P, CPACK_COLS], f32)
        nc.sync.dma_start(
            out=cpak[:, 0:A_COLS], in_=cpack_in[:, 0:A_COLS]
        )
        lhstb = consts.tile([P, NM * 16], bf16)
        LCH = NM * 16 // 2
        for c in range(2):
            nc.sync.dma_start(
                out=lhstb[:, c * LCH : (c + 1) * LCH],
                in_=lhstb_in[:, c * LCH : (c + 1) * LCH],
            )
        nc.sync.dma_start(
            out=cpak[:, A_COLS:CPACK_COLS], in_=cpack_in[:, A_COLS:CPACK_COLS]
        )
        lhsTs = lhstb[:].rearrange("p (j k) -> p j k", k=16)
        biasF = cpak[:, OFF_BIASF : OFF_BIASF + NM]
        biasFn = cpak[:, OFF_BIASFN : OFF_BIASFN + NM]
        ident = cpak[:, OFF_IDENT : OFF_IDENT + P]
        recipD = cpak[:, OFF_RECIPD : OFF_RECIPD + NI * NJ].rearrange(
            "p (I J) -> p I J", J=NJ
        )
        nfac = cpak[:, OFF_NFAC : OFF_NFAC + NI * NJ].rearrange(
            "p (I J) -> p I J", J=NJ
        )
        w1t_a = cpak[:, OFF_W1T : OFF_W1T + 128]
        w1t_b = cpak[:, OFF_W1T + 128 : OFF_W1T + 256]
        w2t_a = cpak[:, OFF_W2T : OFF_W2T + 256]
        w2t_b = cpak[:, OFF_W2T + 256 : OFF_W2T + 512]
        b1c = cpak[:, OFF_B1 : OFF_B1 + 1]
        b2c_a = cpak[:, OFF_B2 : OFF_B2 + 1]
        b2c_b = cpak[:, OFF_B2 + 1 : OFF_B2 + 2]
        emrep = cpak[:, OFF_EM : OFF_EM + NJ]
        biasE = cpak[:, OFF_BIASE : OFF_BIASE + NJ]
        biasEn = cpak[:, OFF_BIASEN : OFF_BIASEN + NJ]
        recipE = cpak[:, OFF_RECIPE : OFF_RECIPE + 1]

        # ---------- big J-loop ----------
        # umm[:, J, 0:64] = per-(i, J, c) biased max; [:, J, 64:128] = biased
        # min -- packed so ONE transpose per J lands ma-feats at partitions
        # 0:64 and mi-feats at 64:128.
        umm = consts.tile([P, NJ, P], f32)
        # z matrices in [feature, row=(I,J)] layout:
        #   rhs_z0: [0:64] m-feats, [64:128] mi-feats
        #   rhs_z1: [0:64] ma-feats, [64:128] std-feats
        rhs_z0 = small.tile([P, P], f32)
        rhs_z1 = small.tile([P, P], f32)
        Ssb = small.tile([16, NJ, P], f32)
        S2sb = small.tile([8, NJ, P], f32)

        with tc.tile_pool(name="psum_sums", bufs=1, space="PSUM") as psum_sums, \
             tc.tile_pool(name="psum_tr", bufs=2, space="PSUM") as psum_tr, \
             tc.tile_pool(name="loop", bufs=3) as loop_pool, \
             tc.tile_pool(name="btmp", bufs=3) as btmp_pool, \
             tc.tile_pool(name="gtmp", bufs=3) as gtmp_pool:
            S_ps = psum_sums.tile([16, NJ, P], f32)

            for J in range(NJ):
                # f32 landing tile [i, c, a] (contiguous DMA, feeds min/max)
                dC = loop_pool.tile([P, D_C, MA], f32, tag="dC")
                nc.sync.dma_start(
                    out=dC[:].rearrange("p c a -> p (c a)"),
                    in_=d_in[:, J, :],
                )
                # bf16 packed [d | d^2] tile for the sums matmul
                dsq = loop_pool.tile([P, P, MA], bf16, tag="dsq")
                nc.scalar.copy(dsq[:, 0:64, :], dC[:])
                nc.scalar.activation(
                    out=dsq[:, 64:128, :], in_=dC[:], func=Act.Square
                )

                # min path: bias-add on GpSimd for J < GMIN, else Vector
                menge = nc.gpsimd if J < GMIN else nc.vector
                mpool = gtmp_pool if J < GMIN else btmp_pool
                bt = mpool.tile([P, D_C, MA], f32, tag="bt")
                menge.tensor_tensor(
                    out=bt,
                    in0=dC[:],
                    in1=biasF[:, J * MA : (J + 1) * MA]
                    .unsqueeze(1)
                    .broadcast_to([P, D_C, MA]),
                    op=Alu.add,
                )
                nc.vector.tensor_reduce(
                    out=umm[:, J, 64:128],
                    in_=bt[:],
                    axis=AX.X,
                    op=Alu.min,
                )
                # max path: bias-add on GpSimd for all J
                bt2 = gtmp_pool.tile([P, D_C, MA], f32, tag="bt2")
                nc.gpsimd.tensor_tensor(
                    out=bt2,
                    in0=dC[:],
                    in1=biasFn[:, J * MA : (J + 1) * MA]
                    .unsqueeze(1)
                    .broadcast_to([P, D_C, MA]),
                    op=Alu.add,
                )
                nc.vector.tensor_reduce(
                    out=umm[:, J, 0:64],
                    in_=bt2[:],
                    axis=AX.X,
                    op=Alu.max,
                )

                # sums: ONE matmul per j, lhsT = [bind*cm1*cm2[j] | bind]
                # (16 cols), rhs = dsq[:, :, jj] = the [d | d^2] column for
                # this a (128 strided cols).  Rows 0:8 = Sm (cols 0:64) and
                # S2 (cols 64:128); rows 8:16 = S1 (cols 0:64).
                for jj in range(MA):
                    j = J * MA + jj
                    nc.tensor.matmul(
                        S_ps[:, J, :],
                        lhsT=lhsTs[:, j, :],
                        rhs=dsq[:, :, jj],
                        start=(jj == 0),
                        stop=(jj == MA - 1),
                    )

            # evacuate sums, duplicated into both 64-column halves so one
            # transpose serves lo and hi features.
            nc.scalar.copy(Ssb[:, :, 0:64], S_ps[0:16, :, 0:64])
            nc.scalar.copy(Ssb[:, :, 64:128], S_ps[0:16, :, 0:64])
            nc.scalar.copy(S2sb[:, :, 0:64], S_ps[0:8, :, 64:128])
            nc.scalar.copy(S2sb[:, :, 64:128], S_ps[0:8, :, 64:128])

            # min/max stage 2: one transpose per J into a PSUM ring of 4,
            # then ONE batched grouped reduce per 4-J round per feature-half.
            for Jr in range(0, NJ, 4):
                TP = psum_tr.tile([P, 4, P], f32, tag="tp")
                for k in range(4):
                    nc.tensor.transpose(
                        out=TP[:, k, :], in_=umm[:, Jr + k, :], identity=ident
                    )
                nc.vector.tensor_reduce(
                    out=rhs_z0[64:128, :]
                    .rearrange("p (I J) -> p J I", J=NJ)[:, Jr : Jr + 4, :],
                    in_=TP[64:128, :, :].rearrange("p r (I a) -> p r I a", a=MA),
                    axis=AX.X,
                    op=Alu.min,
                )
                nc.vector.tensor_reduce(
                    out=rhs_z1[0:64, :]
                    .rearrange("p (I J) -> p J I", J=NJ)[:, Jr : Jr + 4, :],
                    in_=TP[0:64, :, :].rearrange("p r (I a) -> p r I a", a=MA),
                    axis=AX.X,
                    op=Alu.max,
                )

        # ---------- stage 2: sums transposes to [feature, row] layout ----
        with tc.tile_pool(name="psum_ts", bufs=1, space="PSUM") as psum_ts, \
             tc.tile_pool(name="psum_e", bufs=1, space="PSUM") as psum_e, \
             tc.tile_pool(name="psum_o", bufs=1, space="PSUM") as psum_o:
            # sums: one transpose per J: [16, 128(dup)] -> [128, 16]
            # (columns = [SmT | S1T]); rows 0:64 serve m, 64:128 serve std.
            SST_ps = psum_ts.tile([P, 16, NJ], f32, tag="sst")
            S2T_ps = psum_ts.tile([P, NI, NJ], f32, tag="s2t")
            for J in range(NJ):
                nc.tensor.transpose(
                    out=SST_ps[:, :, J],
                    in_=Ssb[0:16, J, :],
                    identity=ident[0:16, 0:16],
                )
                nc.tensor.transpose(
                    out=S2T_ps[:, :, J],
                    in_=S2sb[0:8, J, :],
                    identity=ident[0:8, 0:8],
                )

            SST = small.tile([P, 16, NJ], f32)
            nc.scalar.copy(SST[:], SST_ps[:])
            S2T = small.tile([P, NI, NJ], f32)
            nc.scalar.copy(S2T[64:128], S2T_ps[64:128])
            SmT = SST[:, 0:8, :]
            S1T = SST[:, 8:16, :]

            # m = S1/div  (lo half -> m-features; hi half feeds std)
            mT = small.tile([P, NI, NJ], f32)
            nc.vector.tensor_tensor(out=mT[:], in0=S1T, in1=recipD, op=Alu.mult)
            nc.vector.tensor_copy(
                out=rhs_z0[0:64, :].rearrange("p (I J) -> p I J", J=NJ),
                in_=mT[0:64],
            )
            # std = S2/div - 2*m*(Sm/div) + m^2*nfac     (hi half only)
            A = small.tile([P, NI, NJ], f32)
            nc.vector.tensor_tensor(
                out=A[64:128], in0=S2T[64:128], in1=recipD[64:128], op=Alu.mult
            )
            Bq = small.tile([P, NI, NJ], f32)
            nc.vector.tensor_tensor(
                out=Bq[64:128], in0=SmT[64:128], in1=recipD[64:128], op=Alu.mult
            )
            nc.vector.tensor_tensor(
                out=Bq[64:128], in0=Bq[64:128], in1=mT[64:128], op=Alu.mult
            )
            nc.vector.tensor_scalar(
                Bq[64:128], Bq[64:128], -2.0, None, Alu.mult
            )  # -2*m*Sm/div
            nc.vector.tensor_tensor(
                out=A[64:128], in0=A[64:128], in1=Bq[64:128], op=Alu.add
            )
            Cq = small.tile([P, NI, NJ], f32)
            nc.vector.tensor_tensor(
                out=Cq[64:128], in0=mT[64:128], in1=mT[64:128], op=Alu.mult
            )
            nc.vector.tensor_tensor(
                out=Cq[64:128], in0=Cq[64:128], in1=nfac[64:128], op=Alu.mult
            )
            nc.vector.tensor_tensor(
                out=rhs_z1[64:128, :].rearrange("p (I J) -> p I J", J=NJ),
                in0=A[64:128],
                in1=Cq[64:128],
                op=Alu.add,
            )

            # ---------- E = z @ W1.T + b1 (dup channels on 128 parts) ----
            E_ps = psum_e.tile([P, P], f32)
            nc.tensor.matmul(
                E_ps[:], lhsT=w1t_a, rhs=rhs_z0[:], start=True, stop=False
            )
            nc.tensor.matmul(
                E_ps[:], lhsT=w1t_b, rhs=rhs_z1[:], start=False, stop=True
            )
            E_T = small.tile([P, P], f32)  # [128(dup chan), 128 rows=(I,J)]
            nc.scalar.activation(
                out=E_T[:], in_=E_ps[:], func=Act.Identity, bias=b1c, scale=1.0
            )

            # ---------- stage 2 of the net: masked stats over J ----------
            E_r = E_T[:].rearrange("p (I J) -> p I J", J=NJ)
            zE0 = small.tile([P, NI], f32)  # [0:64] mE, [64:128] miE
            zE1 = small.tile([P, NI], f32)  # [0:64] maE, [64:128] stdE

            # mE (all partitions; lo half is the m-feature, hi feeds stdE)
            mE = small.tile([P, NI], f32)
            nc.vector.tensor_reduce(out=mE[:], in_=E_r, axis=AX.X, op=Alu.add)
            nc.scalar.mul(mE[:], mE[:], recipE)
            nc.scalar.copy(zE0[0:64, :], mE[0:64, :])

            # miE on hi half
            bE = small.tile([P, NI, NJ], f32)
            nc.gpsimd.tensor_tensor(
                out=bE[64:128],
                in0=E_r[64:128],
                in1=biasE[64:128].unsqueeze(1).broadcast_to([64, NI, NJ]),
                op=Alu.add,
            )
            nc.vector.tensor_reduce(
                out=zE0[64:128, :], in_=bE[64:128], axis=AX.X, op=Alu.min
            )
            # maE on lo half (Vector)
            bE2 = small.tile([P, NI, NJ], f32)
            nc.vector.tensor_tensor(
                out=bE2[0:64],
                in0=E_r[0:64],
                in1=biasEn[0:64].unsqueeze(1).broadcast_to([64, NI, NJ]),
                op=Alu.add,
            )
            nc.vector.tensor_reduce(
                out=zE1[0:64, :], in_=bE2[0:64], axis=AX.X, op=Alu.max
            )
            # stdE on hi half: sum(em*(E-mE)^2)/denom  (GpSimd)
            dev = small.tile([P, NI, NJ], f32)
            nc.gpsimd.tensor_tensor(
                out=dev[64:128],
                in0=E_r[64:128],
                in1=mE[64:128].unsqueeze(2).broadcast_to([64, NI, NJ]),
                op=Alu.subtract,
            )
            nc.gpsimd.tensor_tensor(
                out=dev[64:128], in0=dev[64:128], in1=dev[64:128], op=Alu.mult
            )
            nc.gpsimd.tensor_tensor(
                out=dev[64:128],
                in0=dev[64:128],
                in1=emrep[64:128].unsqueeze(1).broadcast_to([64, NI, NJ]),
                op=Alu.mult,
            )
            nc.vector.tensor_reduce(
                out=zE1[64:128, :], in_=dev[64:128], axis=AX.X, op=Alu.add
            )
            nc.scalar.mul(zE1[64:128, :], zE1[64:128, :], recipE[64:128])

            # ---------- out = zE @ W2.T + b2 ----------
            outa_ps = psum_o.tile([128, NI], f32)
            outb_ps = psum_o.tile([128, NI], f32)
            nc.tensor.matmul(
                outa_ps[:], lhsT=w2t_a[:, 0:128], rhs=zE0[:], start=True, stop=False
            )
            nc.tensor.matmul(
                outa_ps[:], lhsT=w2t_b[:, 0:128], rhs=zE1[:], start=False, stop=True
            )
            nc.tensor.matmul(
                outb_ps[:], lhsT=w2t_a[:, 128:256], rhs=zE0[:], start=True, stop=False
            )
            nc.tensor.matmul(
                outb_ps[:], lhsT=w2t_b[:, 128:256], rhs=zE1[:], start=False,
                stop=True,
            )
            outa = small.tile([128, NI], f32)
            nc.scalar.activation(
                out=outa[:], in_=outa_ps[:], func=Act.Identity, bias=b2c_a,
                scale=1.0,
            )
            outb = small.tile([128, NI], f32)
            nc.scalar.activation(
                out=outb[:], in_=outb_ps[:], func=Act.Identity, bias=b2c_b,
                scale=1.0,
            )
            nc.sync.dma_start(out=out_t[0:128, :], in_=outa[:])
            nc.sync.dma_start(out=out_t[128:256, :], in_=outb[:])

    nc.finalize()  # Bacc: runs compile() (wait splitting, reg alloc, ...)
    return nc


def _get_program():
    if "nc" not in _CACHE:
        _CACHE["nc"] = _build_program()
    return _CACHE["nc"]


def _make_in_maps(delta1, c_mask1, c_mask2, e_mask2, W1, b1, W2, b2):
    delta1 = np.asarray(delta1, dtype=np.float32)
    c_mask1 = np.asarray(c_mask1, dtype=np.float32)
    c_mask2 = np.asarray(c_mask2, dtype=np.float32)
    e_mask2 = np.asarray(e_mask2, dtype=np.float32)
    W1 = np.asarray(W1, dtype=np.float32)
    b1 = np.asarray(b1, dtype=np.float32)
    W2 = np.asarray(W2, dtype=np.float32)
    b2 = np.asarray(b2, dtype=np.float32)

    w1t = np.concatenate([W1.T, W1.T], axis=1)  # [256, 128] (dup out-chan)
    w2t = W2.T  # [256, 256]
    bindm = np.zeros((128, 8), dtype=np.float32)
    for i in range(128):
        bindm[i, i // 16] = 1.0
    identm = np.eye(128, dtype=np.float32)

    in_maps = []
    for k in range(8):
        b, ih = k // 2, k % 2
        cm1 = c_mask1[b, ih * 128 : (ih + 1) * 128, 0, 0]        # [128]
        cm2 = c_mask2[b, 0, :, 0]                                 # [256]
        em = e_mask2[b, 0, :, 0]                                  # [16]

        # pre-transpose d to [i, J, c, a]
        dslab = delta1[b, ih * 128 : (ih + 1) * 128]              # [128,256,64]
        dT = np.ascontiguousarray(
            dslab.reshape(128, NJ, MA, D_C).transpose(0, 1, 3, 2)
        ).reshape(128, NJ, D_C * MA)

        cp = np.zeros((128, CPACK_COLS), dtype=np.float32)
        # lhsT fields: [128, 256 j, 16]: cols 0:8 bind*cm1*cm2[j], 8:16 bind
        # (0/1 values -- exact in bf16)
        lhst = np.zeros((128, NM, 16), dtype=np.float32)
        lhst[:, :, 0:8] = (
            bindm[:, None, :] * cm1[:, None, None] * cm2[None, :, None]
        )
        lhst[:, :, 8:16] = bindm[:, None, :]
        lhstb = np.asarray(
            jnp.asarray(lhst.reshape(128, NM * 16), jnp.bfloat16)
        )
        t2 = cm1[:, None] * cm2[None, :]                          # [128, 256]
        cp[:, OFF_BIASF : OFF_BIASF + NM] = BIG * (1.0 - t2)
        cp[:, OFF_BIASFN : OFF_BIASFN + NM] = -BIG * (1.0 - t2)
        cp[:, OFF_IDENT : OFF_IDENT + P] = identm
        cnt1 = bindm.T @ cm1                                      # [8]
        cnt2 = cm2.reshape(NJ, MA).sum(axis=1)                    # [16]
        div = cnt1[:, None] * cnt2[None, :] + EPS                 # [8 I, 16 J]
        cp[:, OFF_RECIPD : OFF_RECIPD + NJ * NI] = (1.0 / div).reshape(-1)[None, :]
        cp[:, OFF_NFAC : OFF_NFAC + NJ * NI] = (1.0 - EPS / div).reshape(-1)[None, :]
        cp[:, OFF_W1T : OFF_W1T + 128] = w1t[0:128, :]
        cp[:, OFF_W1T + 128 : OFF_W1T + 256] = w1t[128:256, :]
        cp[:, OFF_W2T : OFF_W2T + 256] = w2t[0:128, :]
        cp[:, OFF_W2T + 256 : OFF_W2T + 512] = w2t[128:256, :]
        cp[:, OFF_B1] = np.concatenate([b1, b1])
        cp[:, OFF_B2] = b2[0:128]
        cp[:, OFF_B2 + 1] = b2[128:256]
        cp[:, OFF_EM : OFF_EM + NJ] = em[None, :]
        cp[:, OFF_BIASE : OFF_BIASE + NJ] = (BIG * (1.0 - em))[None, :]
        cp[:, OFF_BIASEN : OFF_BIASEN + NJ] = (-BIG * (1.0 - em))[None, :]
        cp[:, OFF_RECIPE] = 1.0 / em.sum()
        in_maps.append(dict(d=dT, cpack=cp, lhstb=lhstb))
    return in_maps


def _assemble(results):
    out = np.empty((4, 16, 256), dtype=np.float32)
    for k in range(8):
        b, ih = k // 2, k % 2
        out[b, ih * 8 : (ih + 1) * 8, :] = results[k]["out_t"].T
    return out


def run(trace=False, **inputs):
    from concourse.bass_utils import run_bass_kernel_spmd

    nc = _get_program()
    in_maps = _make_in_maps(**inputs)
    res = run_bass_kernel_spmd(
        nc, in_maps, core_ids=list(range(8)), trace=trace
    )
    return _assemble(res.results), res


def kernel(**inputs):
    out, _ = run(trace=False, **inputs)
    return out


# revision 33
# speedup vs baseline: 1.5103x; 1.1100x over previous
"""Trainium2 Bass kernel for nn_CtoX (gnn_message_passing).

Computes, per batch b:
  stage1 (CtoE): block-pair stats (mean/min/max/std with pairwise masks) of
     delta1[b] over 16x16 atom blocks -> z[b, 16, 16, 256] -> E = z @ W1.T + b1
  stage2 (EtoX): masked stats of E over its second block axis -> zE[b,16,256]
     -> out = zE @ W2.T + b2   (out: [4, 16, 256])

Sharding: 8 cores = (4 batches) x (2 halves of the first nm axis).
Each core handles delta1[b, ih*128:(ih+1)*128, :, :] (8 MiB) and produces
out[b, ih*8:(ih+1)*8, :] with zero cross-core communication.

v3 layout notes:
  - The host pre-transposes each core's delta1 slice to [128 i, J, c, a]
    (c outer, a inner) so that on-device the bias-add and the grouped
    min/max reduce over `a` are all inner-contiguous (2 elem/cycle DVE
    path) and the per-J DMA lands contiguously (1 descriptor/partition).
  - Per-J working tile dsq[P, 128, MA]: columns 0:64 = d (DMA),
    64:128 = d^2 (Scalar Square).  The sums matmul uses the dsq column
    for one `a` as the STATIONARY tensor and the 16 indicator columns
    [bind*cm1*cm2[j] | bind] as the MOVING tensor, so the accumulated
    result S_psT[(c|c^2), J, (Sm I | S1 I)] comes out with features on
    partitions -- no stage-2 sum transposes needed at all.
  - Bias-adds run on GpSimd for most J (Vector keeps all reduces since
    GpSimd can't reduce over free axes); all mask-derived constants are
    host-precomputed into one cpack tensor.
"""

import numpy as np
import jax.numpy as jnp
from contextlib import ExitStack

BIG = 100000.0
EPS = 1e-8

D_C = 64      # channel dim of delta1
D_X = 256     # output feature dim
MA = 16       # atoms per block
P = 128       # partitions per core (half of nm)
NI = 8        # I-blocks per core
NJ = 16       # J-blocks
NM = 256

GMIN = 14     # J < GMIN: min-path bias-add on GpSimd (else Vector).
              # Max-path bias-adds all run on GpSimd.

# cpack column offsets (one packed [128, CPACK_COLS] constants tensor)
# -- section A (loop-critical first DMA; bf16 lhsT ships separately) --
OFF_BIASF = 0                      # [256] BIG*(1-cm1*cm2)
OFF_BIASFN = OFF_BIASF + NM        # [256] -BIG*(1-cm1*cm2)
A_COLS = OFF_BIASFN + NM
# -- section B (stage-2, last DMA) --
OFF_IDENT = A_COLS                 # [128]
OFF_RECIPD = OFF_IDENT + P         # [8*16] 1/(cnt1*cnt2+eps), (I,J) order
OFF_NFAC = OFF_RECIPD + NJ * NI    # [8*16] 1-EPS/div, (I,J) order
OFF_W1T = OFF_NFAC + NJ * NI       # [256]
OFF_W2T = OFF_W1T + 256            # [512]
OFF_B1 = OFF_W2T + 512             # [1]
OFF_B2 = OFF_B1 + 1                # [2]
OFF_EM = OFF_B2 + 2                # [16]
OFF_BIASE = OFF_EM + NJ            # [16]
OFF_BIASEN = OFF_BIASE + NJ        # [16]
OFF_RECIPE = OFF_BIASEN + NJ       # [1]
CPACK_COLS = OFF_RECIPE + 1

_CACHE = {}


def _build_program():
    import concourse.bass as bass
    import concourse.bacc as bacc
    import concourse.tile as tile
    import concourse.mybir as mybir

    f32 = mybir.dt.float32
    bf16 = mybir.dt.bfloat16
    Alu = mybir.AluOpType
    Act = mybir.ActivationFunctionType
    AX = mybir.AxisListType

    nc = bacc.Bacc()

    # d, host-pretransposed: [i, J, c, a] flattened to [128, NJ, D_C*MA]
    d_in = nc.dram_tensor("d", [P, NJ, D_C * MA], f32, kind="ExternalInput")
    cpack_in = nc.dram_tensor("cpack", [P, CPACK_COLS], f32, kind="ExternalInput")
    # bf16 copy of the lhsT indicator fields (values in {0,1} exact in bf16)
    lhstb_in = nc.dram_tensor("lhstb", [P, NM * 16], bf16, kind="ExternalInput")
    out_t = nc.dram_tensor("out_t", [D_X, NI], f32, kind="ExternalOutput")

    with tile.TileContext(nc) as tc, ExitStack() as ctx:
        consts = ctx.enter_context(tc.tile_pool(name="consts", bufs=1))
        small = ctx.enter_context(tc.tile_pool(name="small", bufs=1))

        # ---------- constant loads: split across DMA queues so the
        # loop-critical constants (biases first, then lhsT chunks in loop
        # order) land fast instead of serializing on one queue ----------
        cpak = consts.tile([P, CPACK_COLS], f32)
        nc.sync.dma_start(
            out=cpak[:, 0:A_COLS], in_=cpack_in[:, 0:A_COLS]
        )
        lhstb = consts.tile([P, NM * 16], bf16)
        LCH = NM * 16 // 2
        for c in range(2):
            nc.sync.dma_start(
                out=lhstb[:, c * LCH : (c + 1) * LCH],
                in_=lhstb_in[:, c * LCH : (c + 1) * LCH],
            )
        nc.sync.dma_start(
            out=cpak[:, A_COLS:CPACK_COLS], in_=cpack_in[:, A_COLS:CPACK_COLS]
        )
        lhsTs = lhstb[:].rearrange("p (j k) -> p j k", k=16)
        biasF = cpak[:, OFF_BIASF : OFF_BIASF + NM]
        biasFn = cpak[:, OFF_BIASFN : OFF_BIASFN + NM]
        ident = cpak[:, OFF_IDENT : OFF_IDENT + P]
        recipD = cpak[:, OFF_RECIPD : OFF_RECIPD + NI * NJ].rearrange(
            "p (I J) -> p I J", J=NJ
        )
        nfac = cpak[:, OFF_NFAC : OFF_NFAC + NI * NJ].rearrange(
            "p (I J) -> p I J", J=NJ
        )
        w1t_a = cpak[:, OFF_W1T : OFF_W1T + 128]
        w1t_b = cpak[:, OFF_W1T + 128 : OFF_W1T + 256]
        w2t_a = cpak[:, OFF_W2T : OFF_W2T + 256]
        w2t_b = cpak[:, OFF_W2T + 256 : OFF_W2T + 512]
        b1c = cpak[:, OFF_B1 : OFF_B1 + 1]
        b2c_a = cpak[:, OFF_B2 : OFF_B2 + 1]
        b2c_b = cpak[:, OFF_B2 + 1 : OFF_B2 + 2]
        emrep = cpak[:, OFF_EM : OFF_EM + NJ]
        biasE = cpak[:, OFF_BIASE : OFF_BIASE + NJ]
        biasEn = cpak[:, OFF_BIASEN : OFF_BIASEN + NJ]
        recipE = cpak[:, OFF_RECIPE : OFF_RECIPE + 1]

        # ---------- big J-loop ----------
        # umm[:, J, 0:64] = per-(i, J, c) biased max; [:, J, 64:128] = biased
        # min -- packed so ONE transpose per J lands ma-feats at partitions
        # 0:64 and mi-feats at 64:128.
        umm = consts.tile([P, NJ, P], f32)
        # z matrices in [feature, row=(I,J)] layout:
        #   rhs_z0: [0:64] m-feats, [64:128] mi-feats
        #   rhs_z1: [0:64] ma-feats, [64:128] std-feats
        rhs_z0 = small.tile([P, P], f32)
        rhs_z1 = small.tile([P, P], f32)
        Ssb = small.tile([16, NJ, P], f32)
        S2sb = small.tile([8, NJ, P], f32)

        with tc.tile_pool(name="psum_sums", bufs=1, space="PSUM") as psum_sums, \
             tc.tile_pool(name="psum_tr", bufs=2, space="PSUM") as psum_tr, \
             tc.tile_pool(name="loop", bufs=3) as loop_pool, \
             tc.tile_pool(name="btmp", bufs=3) as btmp_pool, \
             tc.tile_pool(name="gtmp", bufs=3) as gtmp_pool:
            S_ps = psum_sums.tile([16, NJ, P], f32)

            for J in range(NJ):
                # f32 landing tile [i, a, c] (natural layout, contiguous DMA)
                dC = loop_pool.tile([P, MA, D_C], f32, tag="dC")
                nc.sync.dma_start(
                    out=dC[:].rearrange("p a c -> p (a c)"),
                    in_=d_in[:, J, :],
                )
                # bf16 packed [d | d^2] tile for the sums matmul (layout
                # [a, c2] so each per-jj rhs slice is contiguous)
                dsq = loop_pool.tile([P, MA, P], bf16, tag="dsq")
                nc.scalar.copy(dsq[:, :, 0:64], dC[:])
                nc.scalar.activation(
                    out=dsq[:, :, 64:128], in_=dC[:], func=Act.Square
                )

                # min path: bias-add on GpSimd for J < GMIN, else Vector
                menge = nc.gpsimd if J < GMIN else nc.vector
                mpool = gtmp_pool if J < GMIN else btmp_pool
                bt = mpool.tile([P, MA, D_C], f32, tag="bt")
                menge.tensor_tensor(
                    out=bt,
                    in0=dC[:],
                    in1=biasF[:, J * MA : (J + 1) * MA]
                    .unsqueeze(2)
                    .broadcast_to([P, MA, D_C]),
                    op=Alu.add,
                )
                nc.vector.tensor_reduce(
                    out=umm[:, J, 64:128],
                    in_=bt[:].rearrange("p a c -> p c a"),
                    axis=AX.X,
                    op=Alu.min,
                )
                # max path: bias-add on GpSimd for all J
                bt2 = gtmp_pool.tile([P, MA, D_C], f32, tag="bt2")
                nc.gpsimd.tensor_tensor(
                    out=bt2,
                    in0=dC[:],
                    in1=biasFn[:, J * MA : (J + 1) * MA]
                    .unsqueeze(2)
                    .broadcast_to([P, MA, D_C]),
                    op=Alu.add,
                )
                nc.vector.tensor_reduce(
                    out=umm[:, J, 0:64],
                    in_=bt2[:].rearrange("p a c -> p c a"),
                    axis=AX.X,
                    op=Alu.max,
                )

                # sums: ONE matmul per j, lhsT = [bind*cm1*cm2[j] | bind]
                # (16 cols), rhs = dsq[:, :, jj] = the [d | d^2] column for
                # this a (128 strided cols).  Rows 0:8 = Sm (cols 0:64) and
                # S2 (cols 64:128); rows 8:16 = S1 (cols 0:64).
                for jj in range(MA):
                    j = J * MA + jj
                    nc.tensor.matmul(
                        S_ps[:, J, :],
                        lhsT=lhsTs[:, j, :],
                        rhs=dsq[:, jj, :],
                        start=(jj == 0),
                        stop=(jj == MA - 1),
                    )

            # evacuate sums, duplicated into both 64-column halves so one
            # transpose serves lo and hi features.
            nc.scalar.copy(Ssb[:, :, 0:64], S_ps[0:16, :, 0:64])
            nc.scalar.copy(Ssb[:, :, 64:128], S_ps[0:16, :, 0:64])
            nc.scalar.copy(S2sb[:, :, 0:64], S_ps[0:8, :, 64:128])
            nc.scalar.copy(S2sb[:, :, 64:128], S_ps[0:8, :, 64:128])

            # min/max stage 2: one transpose per J into a PSUM ring of 4,
            # then ONE batched grouped reduce per 4-J round per feature-half.
            for Jr in range(0, NJ, 4):
                TP = psum_tr.tile([P, 4, P], f32, tag="tp")
                for k in range(4):
                    nc.tensor.transpose(
                        out=TP[:, k, :], in_=umm[:, Jr + k, :], identity=ident
                    )
                nc.vector.tensor_reduce(
                    out=rhs_z0[64:128, :]
                    .rearrange("p (I J) -> p J I", J=NJ)[:, Jr : Jr + 4, :],
                    in_=TP[64:128, :, :].rearrange("p r (I a) -> p r I a", a=MA),
                    axis=AX.X,
                    op=Alu.min,
                )
                nc.vector.tensor_reduce(
                    out=rhs_z1[0:64, :]
                    .rearrange("p (I J) -> p J I", J=NJ)[:, Jr : Jr + 4, :],
                    in_=TP[0:64, :, :].rearrange("p r (I a) -> p r I a", a=MA),
                    axis=AX.X,
                    op=Alu.max,
                )

        # ---------- stage 2: sums transposes to [feature, row] layout ----
        with tc.tile_pool(name="psum_ts", bufs=1, space="PSUM") as psum_ts, \
             tc.tile_pool(name="psum_e", bufs=1, space="PSUM") as psum_e, \
             tc.tile_pool(name="psum_o", bufs=1, space="PSUM") as psum_o:
            # sums: one transpose per J: [16, 128(dup)] -> [128, 16]
            # (columns = [SmT | S1T]); rows 0:64 serve m, 64:128 serve std.
            SST_ps = psum_ts.tile([P, 16, NJ], f32, tag="sst")
            S2T_ps = psum_ts.tile([P, NI, NJ], f32, tag="s2t")
            for J in range(NJ):
                nc.tensor.transpose(
                    out=SST_ps[:, :, J],
                    in_=Ssb[0:16, J, :],
                    identity=ident[0:16, 0:16],
                )
                nc.tensor.transpose(
                    out=S2T_ps[:, :, J],
                    in_=S2sb[0:8, J, :],
                    identity=ident[0:8, 0:8],
                )

            SST = small.tile([P, 16, NJ], f32)
            nc.scalar.copy(SST[:], SST_ps[:])
            S2T = small.tile([P, NI, NJ], f32)
            nc.scalar.copy(S2T[64:128], S2T_ps[64:128])
            SmT = SST[:, 0:8, :]
            S1T = SST[:, 8:16, :]

            # m = S1/div  (lo half -> m-features; hi half feeds std)
            mT = small.tile([P, NI, NJ], f32)
            nc.vector.tensor_tensor(out=mT[:], in0=S1T, in1=recipD, op=Alu.mult)
            nc.vector.tensor_copy(
                out=rhs_z0[0:64, :].rearrange("p (I J) -> p I J", J=NJ),
                in_=mT[0:64],
            )
            # std = S2/div - 2*m*(Sm/div) + m^2*nfac     (hi half only)
            A = small.tile([P, NI, NJ], f32)
            nc.vector.tensor_tensor(
                out=A[64:128], in0=S2T[64:128], in1=recipD[64:128], op=Alu.mult
            )
            Bq = small.tile([P, NI, NJ], f32)
            nc.vector.tensor_tensor(
                out=Bq[64:128], in0=SmT[64:128], in1=recipD[64:128], op=Alu.mult
            )
            nc.vector.tensor_tensor(
                out=Bq[64:128], in0=Bq[64:128], in1=mT[64:128], op=Alu.mult
            )
            nc.vector.tensor_scalar(
                Bq[64:128], Bq[64:128], -2.0, None, Alu.mult
            )  # -2*m*Sm/div
            nc.vector.tensor_tensor(
                out=A[64:128], in0=A[64:128], in1=Bq[64:128], op=Alu.add
            )
            Cq = small.tile([P, NI, NJ], f32)
            nc.vector.tensor_tensor(
                out=Cq[64:128], in0=mT[64:128], in1=mT[64:128], op=Alu.mult
            )
            nc.vector.tensor_tensor(
                out=Cq[64:128], in0=Cq[64:128], in1=nfac[64:128], op=Alu.mult
            )
            nc.vector.tensor_tensor(
                out=rhs_z1[64:128, :].rearrange("p (I J) -> p I J", J=NJ),
                in0=A[64:128],
                in1=Cq[64:128],
                op=Alu.add,
            )

            # ---------- E = z @ W1.T + b1 (dup channels on 128 parts) ----
            E_ps = psum_e.tile([P, P], f32)
            nc.tensor.matmul(
                E_ps[:], lhsT=w1t_a, rhs=rhs_z0[:], start=True, stop=False
            )
            nc.tensor.matmul(
                E_ps[:], lhsT=w1t_b, rhs=rhs_z1[:], start=False, stop=True
            )
            E_T = small.tile([P, P], f32)  # [128(dup chan), 128 rows=(I,J)]
            nc.scalar.activation(
                out=E_T[:], in_=E_ps[:], func=Act.Identity, bias=b1c, scale=1.0
            )

            # ---------- stage 2 of the net: masked stats over J ----------
            E_r = E_T[:].rearrange("p (I J) -> p I J", J=NJ)
            zE0 = small.tile([P, NI], f32)  # [0:64] mE, [64:128] miE
            zE1 = small.tile([P, NI], f32)  # [0:64] maE, [64:128] stdE

            # mE (all partitions; lo half is the m-feature, hi feeds stdE)
            mE = small.tile([P, NI], f32)
            nc.vector.tensor_reduce(out=mE[:], in_=E_r, axis=AX.X, op=Alu.add)
            nc.scalar.mul(mE[:], mE[:], recipE)
            nc.scalar.copy(zE0[0:64, :], mE[0:64, :])

            # miE on hi half
            bE = small.tile([P, NI, NJ], f32)
            nc.gpsimd.tensor_tensor(
                out=bE[64:128],
                in0=E_r[64:128],
                in1=biasE[64:128].unsqueeze(1).broadcast_to([64, NI, NJ]),
                op=Alu.add,
            )
            nc.vector.tensor_reduce(
                out=zE0[64:128, :], in_=bE[64:128], axis=AX.X, op=Alu.min
            )
            # maE on lo half (Vector)
            bE2 = small.tile([P, NI, NJ], f32)
            nc.vector.tensor_tensor(
                out=bE2[0:64],
                in0=E_r[0:64],
                in1=biasEn[0:64].unsqueeze(1).broadcast_to([64, NI, NJ]),
                op=Alu.add,
            )
            nc.vector.tensor_reduce(
                out=zE1[0:64, :], in_=bE2[0:64], axis=AX.X, op=Alu.max
            )
            # stdE on hi half: sum(em*(E-mE)^2)/denom  (GpSimd)
            dev = small.tile([P, NI, NJ], f32)
            nc.gpsimd.tensor_tensor(
                out=dev[64:128],
                in0=E_r[64:128],
                in1=mE[64:128].unsqueeze(2).broadcast_to([64, NI, NJ]),
                op=Alu.subtract,
            )
            nc.gpsimd.tensor_tensor(
                out=dev[64:128], in0=dev[64:128], in1=dev[64:128], op=Alu.mult
            )
            nc.gpsimd.tensor_tensor(
                out=dev[64:128],
                in0=dev[64:128],
                in1=emrep[64:128].unsqueeze(1).broadcast_to([64, NI, NJ]),
                op=Alu.mult,
            )
            nc.vector.tensor_reduce(
                out=zE1[64:128, :], in_=dev[64:128], axis=AX.X, op=Alu.add
            )
            nc.scalar.mul(zE1[64:128, :], zE1[64:128, :], recipE[64:128])

            # ---------- out = zE @ W2.T + b2 ----------
            outa_ps = psum_o.tile([128, NI], f32)
            outb_ps = psum_o.tile([128, NI], f32)
            nc.tensor.matmul(
                outa_ps[:], lhsT=w2t_a[:, 0:128], rhs=zE0[:], start=True, stop=False
            )
            nc.tensor.matmul(
                outa_ps[:], lhsT=w2t_b[:, 0:128], rhs=zE1[:], start=False, stop=True
            )
            nc.tensor.matmul(
                outb_ps[:], lhsT=w2t_a[:, 128:256], rhs=zE0[:], start=True, stop=False
            )
            nc.tensor.matmul(
                outb_ps[:], lhsT=w2t_b[:, 128:256], rhs=zE1[:], start=False,
                stop=True,
            )
            outa = small.tile([128, NI], f32)
            nc.scalar.activation(
                out=outa[:], in_=outa_ps[:], func=Act.Identity, bias=b2c_a,
                scale=1.0,
            )
            outb = small.tile([128, NI], f32)
            nc.scalar.activation(
                out=outb[:], in_=outb_ps[:], func=Act.Identity, bias=b2c_b,
                scale=1.0,
            )
            nc.sync.dma_start(out=out_t[0:128, :], in_=outa[:])
            nc.sync.dma_start(out=out_t[128:256, :], in_=outb[:])

    nc.finalize()  # Bacc: runs compile() (wait splitting, reg alloc, ...)
    return nc


def _get_program():
    if "nc" not in _CACHE:
        _CACHE["nc"] = _build_program()
    return _CACHE["nc"]


def _make_in_maps(delta1, c_mask1, c_mask2, e_mask2, W1, b1, W2, b2):
    delta1 = np.asarray(delta1, dtype=np.float32)
    c_mask1 = np.asarray(c_mask1, dtype=np.float32)
    c_mask2 = np.asarray(c_mask2, dtype=np.float32)
    e_mask2 = np.asarray(e_mask2, dtype=np.float32)
    W1 = np.asarray(W1, dtype=np.float32)
    b1 = np.asarray(b1, dtype=np.float32)
    W2 = np.asarray(W2, dtype=np.float32)
    b2 = np.asarray(b2, dtype=np.float32)

    w1t = np.concatenate([W1.T, W1.T], axis=1)  # [256, 128] (dup out-chan)
    w2t = W2.T  # [256, 256]
    bindm = np.zeros((128, 8), dtype=np.float32)
    for i in range(128):
        bindm[i, i // 16] = 1.0
    identm = np.eye(128, dtype=np.float32)

    in_maps = []
    for k in range(8):
        b, ih = k // 2, k % 2
        cm1 = c_mask1[b, ih * 128 : (ih + 1) * 128, 0, 0]        # [128]
        cm2 = c_mask2[b, 0, :, 0]                                 # [256]
        em = e_mask2[b, 0, :, 0]                                  # [16]

        # natural layout [i, J, a, c]
        dT = np.ascontiguousarray(
            delta1[b, ih * 128 : (ih + 1) * 128]
        ).reshape(128, NJ, MA * D_C)

        cp = np.zeros((128, CPACK_COLS), dtype=np.float32)
        # lhsT fields: [128, 256 j, 16]: cols 0:8 bind*cm1*cm2[j], 8:16 bind
        # (0/1 values -- exact in bf16)
        lhst = np.zeros((128, NM, 16), dtype=np.float32)
        lhst[:, :, 0:8] = (
            bindm[:, None, :] * cm1[:, None, None] * cm2[None, :, None]
        )
        lhst[:, :, 8:16] = bindm[:, None, :]
        lhstb = np.asarray(
            jnp.asarray(lhst.reshape(128, NM * 16), jnp.bfloat16)
        )
        t2 = cm1[:, None] * cm2[None, :]                          # [128, 256]
        cp[:, OFF_BIASF : OFF_BIASF + NM] = BIG * (1.0 - t2)
        cp[:, OFF_BIASFN : OFF_BIASFN + NM] = -BIG * (1.0 - t2)
        cp[:, OFF_IDENT : OFF_IDENT + P] = identm
        cnt1 = bindm.T @ cm1                                      # [8]
        cnt2 = cm2.reshape(NJ, MA).sum(axis=1)                    # [16]
        div = cnt1[:, None] * cnt2[None, :] + EPS                 # [8 I, 16 J]
        cp[:, OFF_RECIPD : OFF_RECIPD + NJ * NI] = (1.0 / div).reshape(-1)[None, :]
        cp[:, OFF_NFAC : OFF_NFAC + NJ * NI] = (1.0 - EPS / div).reshape(-1)[None, :]
        cp[:, OFF_W1T : OFF_W1T + 128] = w1t[0:128, :]
        cp[:, OFF_W1T + 128 : OFF_W1T + 256] = w1t[128:256, :]
        cp[:, OFF_W2T : OFF_W2T + 256] = w2t[0:128, :]
        cp[:, OFF_W2T + 256 : OFF_W2T + 512] = w2t[128:256, :]
        cp[:, OFF_B1] = np.concatenate([b1, b1])
        cp[:, OFF_B2] = b2[0:128]
        cp[:, OFF_B2 + 1] = b2[128:256]
        cp[:, OFF_EM : OFF_EM + NJ] = em[None, :]
        cp[:, OFF_BIASE : OFF_BIASE + NJ] = (BIG * (1.0 - em))[None, :]
        cp[:, OFF_BIASEN : OFF_BIASEN + NJ] = (-BIG * (1.0 - em))[None, :]
        cp[:, OFF_RECIPE] = 1.0 / em.sum()
        in_maps.append(dict(d=dT, cpack=cp, lhstb=lhstb))
    return in_maps


def _assemble(results):
    out = np.empty((4, 16, 256), dtype=np.float32)
    for k in range(8):
        b, ih = k // 2, k % 2
        out[b, ih * 8 : (ih + 1) * 8, :] = results[k]["out_t"].T
    return out


def run(trace=False, **inputs):
    from concourse.bass_utils import run_bass_kernel_spmd

    nc = _get_program()
    in_maps = _make_in_maps(**inputs)
    res = run_bass_kernel_spmd(
        nc, in_maps, core_ids=list(range(8)), trace=trace
    )
    return _assemble(res.results), res


def kernel(**inputs):
    out, _ = run(trace=False, **inputs)
    return out


# revision 36
# speedup vs baseline: 1.5424x; 1.0212x over previous
"""Trainium2 Bass kernel for nn_CtoX (gnn_message_passing).

Computes, per batch b:
  stage1 (CtoE): block-pair stats (mean/min/max/std with pairwise masks) of
     delta1[b] over 16x16 atom blocks -> z[b, 16, 16, 256] -> E = z @ W1.T + b1
  stage2 (EtoX): masked stats of E over its second block axis -> zE[b,16,256]
     -> out = zE @ W2.T + b2   (out: [4, 16, 256])

Sharding: 8 cores = (4 batches) x (2 halves of the first nm axis).
Each core handles delta1[b, ih*128:(ih+1)*128, :, :] (8 MiB) and produces
out[b, ih*8:(ih+1)*8, :] with zero cross-core communication.

v3 layout notes:
  - The host pre-transposes each core's delta1 slice to [128 i, J, c, a]
    (c outer, a inner) so that on-device the bias-add and the grouped
    min/max reduce over `a` are all inner-contiguous (2 elem/cycle DVE
    path) and the per-J DMA lands contiguously (1 descriptor/partition).
  - Per-J working tile dsq[P, 128, MA]: columns 0:64 = d (DMA),
    64:128 = d^2 (Scalar Square).  The sums matmul uses the dsq column
    for one `a` as the STATIONARY tensor and the 16 indicator columns
    [bind*cm1*cm2[j] | bind] as the MOVING tensor, so the accumulated
    result S_psT[(c|c^2), J, (Sm I | S1 I)] comes out with features on
    partitions -- no stage-2 sum transposes needed at all.
  - Bias-adds run on GpSimd for most J (Vector keeps all reduces since
    GpSimd can't reduce over free axes); all mask-derived constants are
    host-precomputed into one cpack tensor.
"""

import numpy as np
import jax.numpy as jnp
from contextlib import ExitStack

BIG = 100000.0
EPS = 1e-8

D_C = 64      # channel dim of delta1
D_X = 256     # output feature dim
MA = 16       # atoms per block
P = 128       # partitions per core (half of nm)
NI = 8        # I-blocks per core
NJ = 16       # J-blocks
NM = 256

VMIN = 2      # J < VMIN: min-path bias-add on Vector (ramp: vector would
              # otherwise idle waiting on gpsimd at loop start); GpSimd
              # takes the rest.  Max-path bias-adds all run on GpSimd.

# cpack column offsets (one packed [128, CPACK_COLS] constants tensor)
# -- section A (loop-critical first DMA; bf16 lhsT ships separately) --
OFF_BIASF = 0                      # [256] BIG*(1-cm1*cm2)
OFF_BIASFN = OFF_BIASF + NM        # [256] -BIG*(1-cm1*cm2)
A_COLS = OFF_BIASFN + NM
# -- section B (stage-2, last DMA) --
OFF_IDENT = A_COLS                 # [128]
OFF_RECIPD = OFF_IDENT + P         # [8*16] 1/(cnt1*cnt2+eps), (I,J) order
OFF_NFAC = OFF_RECIPD + NJ * NI    # [8*16] 1-EPS/div, (I,J) order
OFF_W1T = OFF_NFAC + NJ * NI       # [256]
OFF_W2T = OFF_W1T + 256            # [512]
OFF_B1 = OFF_W2T + 512             # [1]
OFF_B2 = OFF_B1 + 1                # [2]
OFF_EM = OFF_B2 + 2                # [16]
OFF_BIASE = OFF_EM + NJ            # [16]
OFF_BIASEN = OFF_BIASE + NJ        # [16]
OFF_RECIPE = OFF_BIASEN + NJ       # [1]
CPACK_COLS = OFF_RECIPE + 1

_CACHE = {}


def _build_program():
    import concourse.bass as bass
    import concourse.bacc as bacc
    import concourse.tile as tile
    import concourse.mybir as mybir

    f32 = mybir.dt.float32
    bf16 = mybir.dt.bfloat16
    Alu = mybir.AluOpType
    Act = mybir.ActivationFunctionType
    AX = mybir.AxisListType

    nc = bacc.Bacc()

    # d, host-pretransposed: [i, J, c, a] flattened to [128, NJ, D_C*MA]
    d_in = nc.dram_tensor("d", [P, NJ, D_C * MA], f32, kind="ExternalInput")
    cpack_in = nc.dram_tensor("cpack", [P, CPACK_COLS], f32, kind="ExternalInput")
    # bf16 copy of the lhsT indicator fields (values in {0,1} exact in bf16)
    lhstb_in = nc.dram_tensor("lhstb", [P, NM * 16], bf16, kind="ExternalInput")
    out_t = nc.dram_tensor("out_t", [D_X, NI], f32, kind="ExternalOutput")

    with tile.TileContext(nc) as tc, ExitStack() as ctx:
        consts = ctx.enter_context(tc.tile_pool(name="consts", bufs=1))
        small = ctx.enter_context(tc.tile_pool(name="small", bufs=1))

        # ---------- constant loads: split across DMA queues so the
        # loop-critical constants (biases first, then lhsT chunks in loop
        # order) land fast instead of serializing on one queue ----------
        cpak = consts.tile([P, CPACK_COLS], f32)
        nc.sync.dma_start(
            out=cpak[:, 0:A_COLS], in_=cpack_in[:, 0:A_COLS]
        )
        lhstb = consts.tile([P, NM * 16], bf16)
        LCH = NM * 16 // 2
        for c in range(2):
            nc.sync.dma_start(
                out=lhstb[:, c * LCH : (c + 1) * LCH],
                in_=lhstb_in[:, c * LCH : (c + 1) * LCH],
            )
        nc.sync.dma_start(
            out=cpak[:, A_COLS:CPACK_COLS], in_=cpack_in[:, A_COLS:CPACK_COLS]
        )
        lhsTs = lhstb[:].rearrange("p (j k) -> p j k", k=16)
        biasF = cpak[:, OFF_BIASF : OFF_BIASF + NM]
        biasFn = cpak[:, OFF_BIASFN : OFF_BIASFN + NM]
        ident = cpak[:, OFF_IDENT : OFF_IDENT + P]
        recipD = cpak[:, OFF_RECIPD : OFF_RECIPD + NI * NJ].rearrange(
            "p (I J) -> p I J", J=NJ
        )
        nfac = cpak[:, OFF_NFAC : OFF_NFAC + NI * NJ].rearrange(
            "p (I J) -> p I J", J=NJ
        )
        w1t_a = cpak[:, OFF_W1T : OFF_W1T + 128]
        w1t_b = cpak[:, OFF_W1T + 128 : OFF_W1T + 256]
        w2t_a = cpak[:, OFF_W2T : OFF_W2T + 256]
        w2t_b = cpak[:, OFF_W2T + 256 : OFF_W2T + 512]
        b1c = cpak[:, OFF_B1 : OFF_B1 + 1]
        b2c_a = cpak[:, OFF_B2 : OFF_B2 + 1]
        b2c_b = cpak[:, OFF_B2 + 1 : OFF_B2 + 2]
        emrep = cpak[:, OFF_EM : OFF_EM + NJ]
        biasE = cpak[:, OFF_BIASE : OFF_BIASE + NJ]
        biasEn = cpak[:, OFF_BIASEN : OFF_BIASEN + NJ]
        recipE = cpak[:, OFF_RECIPE : OFF_RECIPE + 1]

        # ---------- big J-loop ----------
        # umm[:, J, 0:64] = per-(i, J, c) biased max; [:, J, 64:128] = biased
        # min -- packed so ONE transpose per J lands ma-feats at partitions
        # 0:64 and mi-feats at 64:128.
        umm = consts.tile([P, NJ, P], f32)
        # z matrices in [feature, row=(I,J)] layout:
        #   rhs_z0: [0:64] m-feats, [64:128] mi-feats
        #   rhs_z1: [0:64] ma-feats, [64:128] std-feats
        rhs_z0 = small.tile([P, P], f32)
        rhs_z1 = small.tile([P, P], f32)
        Ssb = small.tile([16, NJ, P], f32)
        S2sb = small.tile([8, NJ, P], f32)

        with tc.tile_pool(name="psum_sums", bufs=1, space="PSUM") as psum_sums, \
             tc.tile_pool(name="psum_tr", bufs=2, space="PSUM") as psum_tr, \
             tc.tile_pool(name="loop", bufs=3) as loop_pool, \
             tc.tile_pool(name="btmp", bufs=3) as btmp_pool, \
             tc.tile_pool(name="gtmp", bufs=3) as gtmp_pool:
            S_ps = psum_sums.tile([16, NJ, P], f32)

            for J in range(NJ):
                # f32 landing tile [i, a, c] (natural layout, contiguous DMA)
                dC = loop_pool.tile([P, MA, D_C], f32, tag="dC")
                nc.sync.dma_start(
                    out=dC[:].rearrange("p a c -> p (a c)"),
                    in_=d_in[:, J, :],
                )
                # bf16 packed [d | d^2] tile for the sums matmul (layout
                # [a, c2] so each per-jj rhs slice is contiguous)
                dsq = loop_pool.tile([P, MA, P], bf16, tag="dsq")
                nc.scalar.copy(dsq[:, :, 0:64], dC[:])
                nc.scalar.activation(
                    out=dsq[:, :, 64:128], in_=dC[:], func=Act.Square
                )

                # min path: bias-add on Vector for J < VMIN, else GpSimd
                menge = nc.vector if J < VMIN else nc.gpsimd
                mpool = btmp_pool if J < VMIN else gtmp_pool
                bt = mpool.tile([P, MA, D_C], f32, tag="bt")
                menge.tensor_tensor(
                    out=bt,
                    in0=dC[:],
                    in1=biasF[:, J * MA : (J + 1) * MA]
                    .unsqueeze(2)
                    .broadcast_to([P, MA, D_C]),
                    op=Alu.add,
                )
                nc.vector.tensor_reduce(
                    out=umm[:, J, 64:128],
                    in_=bt[:].rearrange("p a c -> p c a"),
                    axis=AX.X,
                    op=Alu.min,
                )
                # max path: bias-add on GpSimd for all J
                bt2 = gtmp_pool.tile([P, MA, D_C], f32, tag="bt2")
                nc.gpsimd.tensor_tensor(
                    out=bt2,
                    in0=dC[:],
                    in1=biasFn[:, J * MA : (J + 1) * MA]
                    .unsqueeze(2)
                    .broadcast_to([P, MA, D_C]),
                    op=Alu.add,
                )
                nc.vector.tensor_reduce(
                    out=umm[:, J, 0:64],
                    in_=bt2[:].rearrange("p a c -> p c a"),
                    axis=AX.X,
                    op=Alu.max,
                )

                # sums: ONE matmul per j, lhsT = [bind*cm1*cm2[j] | bind]
                # (16 cols), rhs = dsq[:, :, jj] = the [d | d^2] column for
                # this a (128 strided cols).  Rows 0:8 = Sm (cols 0:64) and
                # S2 (cols 64:128); rows 8:16 = S1 (cols 0:64).
                for jj in range(MA):
                    j = J * MA + jj
                    nc.tensor.matmul(
                        S_ps[:, J, :],
                        lhsT=lhsTs[:, j, :],
                        rhs=dsq[:, jj, :],
                        start=(jj == 0),
                        stop=(jj == MA - 1),
                    )

            # evacuate sums, duplicated into both 64-column halves so one
            # transpose serves lo and hi features.
            nc.scalar.copy(Ssb[:, :, 0:64], S_ps[0:16, :, 0:64])
            nc.scalar.copy(Ssb[:, :, 64:128], S_ps[0:16, :, 0:64])
            nc.scalar.copy(S2sb[:, :, 0:64], S_ps[0:8, :, 64:128])
            nc.scalar.copy(S2sb[:, :, 64:128], S_ps[0:8, :, 64:128])

            # min/max stage 2: one transpose per J into a PSUM ring of 4,
            # then ONE batched grouped reduce per 4-J round per feature-half.
            for Jr in range(0, NJ, 4):
                TP = psum_tr.tile([P, 4, P], f32, tag="tp")
                for k in range(4):
                    nc.tensor.transpose(
                        out=TP[:, k, :], in_=umm[:, Jr + k, :], identity=ident
                    )
                nc.vector.tensor_reduce(
                    out=rhs_z0[64:128, :]
                    .rearrange("p (I J) -> p J I", J=NJ)[:, Jr : Jr + 4, :],
                    in_=TP[64:128, :, :].rearrange("p r (I a) -> p r I a", a=MA),
                    axis=AX.X,
                    op=Alu.min,
                )
                nc.vector.tensor_reduce(
                    out=rhs_z1[0:64, :]
                    .rearrange("p (I J) -> p J I", J=NJ)[:, Jr : Jr + 4, :],
                    in_=TP[0:64, :, :].rearrange("p r (I a) -> p r I a", a=MA),
                    axis=AX.X,
                    op=Alu.max,
                )

        # ---------- stage 2: sums transposes to [feature, row] layout ----
        with tc.tile_pool(name="psum_ts", bufs=1, space="PSUM") as psum_ts, \
             tc.tile_pool(name="psum_e", bufs=1, space="PSUM") as psum_e, \
             tc.tile_pool(name="psum_o", bufs=1, space="PSUM") as psum_o:
            # sums: one transpose per J: [16, 128(dup)] -> [128, 16]
            # (columns = [SmT | S1T]); rows 0:64 serve m, 64:128 serve std.
            SST_ps = psum_ts.tile([P, 16, NJ], f32, tag="sst")
            S2T_ps = psum_ts.tile([P, NI, NJ], f32, tag="s2t")
            for J in range(NJ):
                nc.tensor.transpose(
                    out=SST_ps[:, :, J],
                    in_=Ssb[0:16, J, :],
                    identity=ident[0:16, 0:16],
                )
                nc.tensor.transpose(
                    out=S2T_ps[:, :, J],
                    in_=S2sb[0:8, J, :],
                    identity=ident[0:8, 0:8],
                )

            SST = small.tile([P, 16, NJ], f32)
            nc.scalar.copy(SST[:], SST_ps[:])
            S2T = small.tile([P, NI, NJ], f32)
            nc.scalar.copy(S2T[64:128], S2T_ps[64:128])
            SmT = SST[:, 0:8, :]
            S1T = SST[:, 8:16, :]

            # m = S1/div  (lo half -> m-features; hi half feeds std)
            mT = small.tile([P, NI, NJ], f32)
            nc.vector.tensor_tensor(out=mT[:], in0=S1T, in1=recipD, op=Alu.mult)
            nc.vector.tensor_copy(
                out=rhs_z0[0:64, :].rearrange("p (I J) -> p I J", J=NJ),
                in_=mT[0:64],
            )
            # std = S2/div - 2*m*(Sm/div) + m^2*nfac     (hi half only)
            A = small.tile([P, NI, NJ], f32)
            nc.vector.tensor_tensor(
                out=A[64:128], in0=S2T[64:128], in1=recipD[64:128], op=Alu.mult
            )
            Bq = small.tile([P, NI, NJ], f32)
            nc.vector.tensor_tensor(
                out=Bq[64:128], in0=SmT[64:128], in1=recipD[64:128], op=Alu.mult
            )
            nc.vector.tensor_tensor(
                out=Bq[64:128], in0=Bq[64:128], in1=mT[64:128], op=Alu.mult
            )
            nc.vector.tensor_scalar(
                Bq[64:128], Bq[64:128], -2.0, None, Alu.mult
            )  # -2*m*Sm/div
            nc.vector.tensor_tensor(
                out=A[64:128], in0=A[64:128], in1=Bq[64:128], op=Alu.add
            )
            Cq = small.tile([P, NI, NJ], f32)
            nc.vector.tensor_tensor(
                out=Cq[64:128], in0=mT[64:128], in1=mT[64:128], op=Alu.mult
            )
            nc.vector.tensor_tensor(
                out=Cq[64:128], in0=Cq[64:128], in1=nfac[64:128], op=Alu.mult
            )
            nc.vector.tensor_tensor(
                out=rhs_z1[64:128, :].rearrange("p (I J) -> p I J", J=NJ),
                in0=A[64:128],
                in1=Cq[64:128],
                op=Alu.add,
            )

            # ---------- E = z @ W1.T + b1 (dup channels on 128 parts) ----
            E_ps = psum_e.tile([P, P], f32)
            nc.tensor.matmul(
                E_ps[:], lhsT=w1t_a, rhs=rhs_z0[:], start=True, stop=False
            )
            nc.tensor.matmul(
                E_ps[:], lhsT=w1t_b, rhs=rhs_z1[:], start=False, stop=True
            )
            E_T = small.tile([P, P], f32)  # [128(dup chan), 128 rows=(I,J)]
            nc.scalar.activation(
                out=E_T[:], in_=E_ps[:], func=Act.Identity, bias=b1c, scale=1.0
            )

            # ---------- stage 2 of the net: masked stats over J ----------
            E_r = E_T[:].rearrange("p (I J) -> p I J", J=NJ)
            zE0 = small.tile([P, NI], f32)  # [0:64] mE, [64:128] miE
            zE1 = small.tile([P, NI], f32)  # [0:64] maE, [64:128] stdE

            # mE (all partitions; lo half is the m-feature, hi feeds stdE)
            mE = small.tile([P, NI], f32)
            nc.vector.tensor_reduce(out=mE[:], in_=E_r, axis=AX.X, op=Alu.add)
            nc.scalar.mul(mE[:], mE[:], recipE)
            nc.scalar.copy(zE0[0:64, :], mE[0:64, :])

            # miE on hi half
            bE = small.tile([P, NI, NJ], f32)
            nc.gpsimd.tensor_tensor(
                out=bE[64:128],
                in0=E_r[64:128],
                in1=biasE[64:128].unsqueeze(1).broadcast_to([64, NI, NJ]),
                op=Alu.add,
            )
            nc.vector.tensor_reduce(
                out=zE0[64:128, :], in_=bE[64:128], axis=AX.X, op=Alu.min
            )
            # maE on lo half (add on GpSimd, reduce on Vector)
            bE2 = small.tile([P, NI, NJ], f32)
            nc.gpsimd.tensor_tensor(
                out=bE2[0:64],
                in0=E_r[0:64],
                in1=biasEn[0:64].unsqueeze(1).broadcast_to([64, NI, NJ]),
                op=Alu.add,
            )
            nc.vector.tensor_reduce(
                out=zE1[0:64, :], in_=bE2[0:64], axis=AX.X, op=Alu.max
            )
            # stdE on hi half: sum(em*(E-mE)^2)/denom  (GpSimd)
            dev = small.tile([P, NI, NJ], f32)
            nc.gpsimd.tensor_tensor(
                out=dev[64:128],
                in0=E_r[64:128],
                in1=mE[64:128].unsqueeze(2).broadcast_to([64, NI, NJ]),
                op=Alu.subtract,
            )
            nc.gpsimd.tensor_tensor(
                out=dev[64:128], in0=dev[64:128], in1=dev[64:128], op=Alu.mult
            )
            nc.gpsimd.tensor_tensor(
                out=dev[64:128],
                in0=dev[64:128],
                in1=emrep[64:128].unsqueeze(1).broadcast_to([64, NI, NJ]),
                op=Alu.mult,
            )
            nc.vector.tensor_reduce(
                out=zE1[64:128, :], in_=dev[64:128], axis=AX.X, op=Alu.add
            )
            nc.scalar.mul(zE1[64:128, :], zE1[64:128, :], recipE[64:128])

            # ---------- out = zE @ W2.T + b2 ----------
            outa_ps = psum_o.tile([128, NI], f32)
            outb_ps = psum_o.tile([128, NI], f32)
            nc.tensor.matmul(
                outa_ps[:], lhsT=w2t_a[:, 0:128], rhs=zE0[:], start=True, stop=False
            )
            nc.tensor.matmul(
                outa_ps[:], lhsT=w2t_b[:, 0:128], rhs=zE1[:], start=False, stop=True
            )
            nc.tensor.matmul(
                outb_ps[:], lhsT=w2t_a[:, 128:256], rhs=zE0[:], start=True, stop=False
            )
            nc.tensor.matmul(
                outb_ps[:], lhsT=w2t_b[:, 128:256], rhs=zE1[:], start=False,
                stop=True,
            )
            outa = small.tile([128, NI], f32)
            nc.scalar.activation(
                out=outa[:], in_=outa_ps[:], func=Act.Identity, bias=b2c_a,
                scale=1.0,
            )
            outb = small.tile([128, NI], f32)
            nc.scalar.activation(
                out=outb[:], in_=outb_ps[:], func=Act.Identity, bias=b2c_b,
                scale=1.0,
            )
            nc.sync.dma_start(out=out_t[0:128, :], in_=outa[:])
            nc.sync.dma_start(out=out_t[128:256, :], in_=outb[:])

    nc.finalize()  # Bacc: runs compile() (wait splitting, reg alloc, ...)
    return nc


def _get_program():
    if "nc" not in _CACHE:
        _CACHE["nc"] = _build_program()
    return _CACHE["nc"]


def _make_in_maps(delta1, c_mask1, c_mask2, e_mask2, W1, b1, W2, b2):
    delta1 = np.asarray(delta1, dtype=np.float32)
    c_mask1 = np.asarray(c_mask1, dtype=np.float32)
    c_mask2 = np.asarray(c_mask2, dtype=np.float32)
    e_mask2 = np.asarray(e_mask2, dtype=np.float32)
    W1 = np.asarray(W1, dtype=np.float32)
    b1 = np.asarray(b1, dtype=np.float32)
    W2 = np.asarray(W2, dtype=np.float32)
    b2 = np.asarray(b2, dtype=np.float32)

    w1t = np.concatenate([W1.T, W1.T], axis=1)  # [256, 128] (dup out-chan)
    w2t = W2.T  # [256, 256]
    bindm = np.zeros((128, 8), dtype=np.float32)
    for i in range(128):
        bindm[i, i // 16] = 1.0
    identm = np.eye(128, dtype=np.float32)

    in_maps = []
    for k in range(8):
        b, ih = k // 2, k % 2
        cm1 = c_mask1[b, ih * 128 : (ih + 1) * 128, 0, 0]        # [128]
        cm2 = c_mask2[b, 0, :, 0]                                 # [256]
        em = e_mask2[b, 0, :, 0]                                  # [16]

        # natural layout [i, J, a, c]
        dT = np.ascontiguousarray(
            delta1[b, ih * 128 : (ih + 1) * 128]
        ).reshape(128, NJ, MA * D_C)

        cp = np.zeros((128, CPACK_COLS), dtype=np.float32)
        # lhsT fields: [128, 256 j, 16]: cols 0:8 bind*cm1*cm2[j], 8:16 bind
        # (0/1 values -- exact in bf16)
        lhst = np.zeros((128, NM, 16), dtype=np.float32)
        lhst[:, :, 0:8] = (
            bindm[:, None, :] * cm1[:, None, None] * cm2[None, :, None]
        )
        lhst[:, :, 8:16] = bindm[:, None, :]
        lhstb = np.asarray(
            jnp.asarray(lhst.reshape(128, NM * 16), jnp.bfloat16)
        )
        t2 = cm1[:, None] * cm2[None, :]                          # [128, 256]
        cp[:, OFF_BIASF : OFF_BIASF + NM] = BIG * (1.0 - t2)
        cp[:, OFF_BIASFN : OFF_BIASFN + NM] = -BIG * (1.0 - t2)
        cp[:, OFF_IDENT : OFF_IDENT + P] = identm
        cnt1 = bindm.T @ cm1                                      # [8]
        cnt2 = cm2.reshape(NJ, MA).sum(axis=1)                    # [16]
        div = cnt1[:, None] * cnt2[None, :] + EPS                 # [8 I, 16 J]
        cp[:, OFF_RECIPD : OFF_RECIPD + NJ * NI] = (1.0 / div).reshape(-1)[None, :]
        cp[:, OFF_NFAC : OFF_NFAC + NJ * NI] = (1.0 - EPS / div).reshape(-1)[None, :]
        cp[:, OFF_W1T : OFF_W1T + 128] = w1t[0:128, :]
        cp[:, OFF_W1T + 128 : OFF_W1T + 256] = w1t[128:256, :]
        cp[:, OFF_W2T : OFF_W2T + 256] = w2t[0:128, :]
        cp[:, OFF_W2T + 256 : OFF_W2T + 512] = w2t[128:256, :]
        cp[:, OFF_B1] = np.concatenate([b1, b1])
        cp[:, OFF_B2] = b2[0:128]
        cp[:, OFF_B2 + 1] = b2[128:256]
        cp[:, OFF_EM : OFF_EM + NJ] = em[None, :]
        cp[:, OFF_BIASE : OFF_BIASE + NJ] = (BIG * (1.0 - em))[None, :]
        cp[:, OFF_BIASEN : OFF_BIASEN + NJ] = (-BIG * (1.0 - em))[None, :]
        cp[:, OFF_RECIPE] = 1.0 / em.sum()
        in_maps.append(dict(d=dT, cpack=cp, lhstb=lhstb))
    return in_maps


def _assemble(results):
    out = np.empty((4, 16, 256), dtype=np.float32)
    for k in range(8):
        b, ih = k // 2, k % 2
        out[b, ih * 8 : (ih + 1) * 8, :] = results[k]["out_t"].T
    return out


def run(trace=False, **inputs):
    from concourse.bass_utils import run_bass_kernel_spmd

    nc = _get_program()
    in_maps = _make_in_maps(**inputs)
    res = run_bass_kernel_spmd(
        nc, in_maps, core_ids=list(range(8)), trace=trace
    )
    return _assemble(res.results), res


def kernel(**inputs):
    out, _ = run(trace=False, **inputs)
    return out
